# revision 1
# baseline (speedup 1.0000x reference)
import numpy as np

N=4096; C=1024; INTER=128; R=128; RR=R*R; GC=256; NCORES=8; NB=N//NCORES
PW=130; HR=R//NCORES           # 16 output h-rows per core
WINR=HR+2                      # 18 padded rows in window
WIN=WINR*PW                    # 2340
QT=(WIN+127)//128              # 19 k-tiles for q
QPAD=QT*128                    # 2432
PWIN=2694                      # window read span
PGLOB=17280                    # padded p buffer (guard 131 + 16900 + tail)
AGS=NB*(INTER+1)+HR*R          # 512*129+2048 = 68096
ARS=9*GC+C                     # 2304+1024 = 3328
KT=C//128                      # 8

_cache = {}

def _fold(p):
    f32=np.float32
    out={}
    mcw1=p['m_cw'][:INTER]; mcw2=p['m_cw'][INTER:]
    xv=np.zeros((C,6),f32); sc=np.zeros((1,8),f32)
    xv[:,0]=p['m_tw'].T@mcw1; sc[0,0]=p['m_tb']@mcw1            # a
    for j in range(3):
        c1=p['pr_cw'][j,:INTER]; c2=p['pr_cw'][j,INTER:]
        xv[:,1+j]=p['pr_tw'][j].T@c1
        sc[0,1+j]=p['pr_tb'][j]@c1+p['pr_pb'][j]@c2
    xv[:,4]=p['ba_tw'].T@p['ba_cw'][:INTER]
    xv[:,5]=p['m_pw'].T@mcw2; sc[0,5]=p['m_pb']@mcw2            # b
    sc[0,4]=p['ba_tb']@p['ba_cw'][:INTER]+p['ba_pb']@p['ba_cw'][INTER:]
    out['xvecs']=xv; out['sconst']=sc
    vps=np.stack([p['pr_pw'][j].T@p['pr_cw'][j,INTER:] for j in range(3)],1)
    out['vps']=vps.astype(f32)                                   # [C,3]
    out['vpm']=(p['ba_pw'].T@p['ba_cw'][INTER:]/ (2*N)).astype(f32)[:,None]  # [C,1]
    out['m_gwT']=p['m_gw'].T.copy()                              # [C,128]
    out['pr_gwT']=np.stack([p['pr_gw'][j].T for j in range(3)])  # [3,C,128]
    bg=float(p['ba_g'][0])
    out['ba_gwT']=(bg*p['ba_gw'].T/(2*N)).copy()                 # [C,128]
    sg=float(p['sp_g'][0])
    # sp_gwT: [(mh*3+mw)*GC+ic, oc] with (kh,kw)=(2-mh,2-mw), scaled by sp_g
    g=np.transpose(p['sp_gw'],(2,3,1,0))[::-1,::-1]              # [kh',kw',ic,oc] reversed
    out['sp_gwT']=np.ascontiguousarray(sg*g.reshape(9*GC,INTER))
    # w_effT [2,128,9]: w_eff[ic,kh,kw]=sum_c spcw2[c]*sp_pw[c,ic,kh,kw]
    we=np.einsum('c,cikl->ikl',p['sp_cw'][INTER:],p['sp_pw'])    # [GC,3,3]
    out['w_effT']=we.reshape(2,128,9).astype(f32)
    # biases128 [128,6]: m_gb, pr_gb0..2, ba_g*ba_gb(gm bias), sp_g*sp_gb(v bias)
    b6=np.zeros((INTER,6),f32)
    b6[:,0]=p['m_gb']; b6[:,1:4]=p['pr_gb'].T; b6[:,4]=bg*p['ba_gb']; b6[:,5]=sg*p['sp_gb']
    out['bias128']=b6
    gf=np.zeros((1,4*INTER),f32)
    for j in range(3): gf[0,j*INTER:(j+1)*INTER]=p['pr_g'][j]
    gf[0,3*INTER:]=1.0
    out['gfill']=gf
    out['mgb_row']=p['m_gb'][None,:].astype(f32)                 # [1,128] K=1 bias trick
    return out

def _shard(p):
    f32=np.float32
    gpadded=np.pad(p['global_feature'][0],((0,0),(1,1),(1,1)))   # [GC,130,130]
    ins=[]
    for k in range(NCORES):
        d={}
        rs=slice(k*NB,(k+1)*NB)
        d['xT']=np.ascontiguousarray(p['origin_feature'][rs].T)
        yt=np.stack([np.ascontiguousarray(t[rs].T) for t in
                     (p['local_feature'],p['bef_l'],p['aft_l'])])
        d['yT']=yt                                               # [3,C,NB]
        d['bafT']=np.ascontiguousarray(np.concatenate(
            [p['bef'][rs],p['aft'][rs]],0).T)                    # [C,2NB]
        gw=gpadded[:,k*HR:k*HR+WINR,:]                           # [GC,18,130]
        d['gpad']=np.ascontiguousarray(gw.reshape(2,128,WINR*PW)
                    .transpose(1,0,2).reshape(128,2*WINR*PW))
        gt=np.zeros((QPAD,GC),f32)
        gt[:WIN]=gw.reshape(GC,WIN).T
        d['gpadT']=gt.reshape(QT,128,GC)
        osel=np.zeros((NCORES,1),f32); osel[k,0]=1.0
        d['osel']=osel
        ins.append(d)
    return ins

def kernel(**inputs):
    import ml_dtypes  # noqa
    if 'nc' not in _cache:
        _cache['nc']=build()
    nc=_cache['nc']
    fold=_fold(inputs); shards=_shard(inputs)
    in_maps=[]
    for k in range(NCORES):
        m=dict(shards[k]); m.update(fold)
        in_maps.append({kk:np.ascontiguousarray(v,dtype=np.float32) for kk,v in m.items()})
    from concourse.bass_utils import run_bass_kernel_spmd
    res=run_bass_kernel_spmd(nc,in_maps,list(range(NCORES)))
    out=np.empty((N,INTER),np.float32)
    for k in range(NCORES):
        out[k*NB:(k+1)*NB]=res.results[k]['out'].T
    return out


# ---- device program builder (inlined) ----
import numpy as np
import bass_rust
import concourse.bass as bass
import concourse.bacc as bacc
import concourse.mybir as mybir
import concourse.tile as tile

F32=mybir.dt.float32
AF=mybir.ActivationFunctionType
AL=mybir.AluOpType
RG=[list(range(NCORES))]

def mkap(a,offset,dims):
    b=a.copy(); b.offset=offset
    b.ap=bass_rust.VecI64Pair([list(d) for d in dims])
    return b

def build():
    nc=bacc.Bacc("TRN2",target_bir_lowering=False,debug=False,num_devices=NCORES)
    P=lambda n,s: nc.declare_dram_parameter(n,list(s),F32,isOutput=False)
    xT=P('xT',(C,NB)); yT=P('yT',(3,C,NB)); bafT=P('bafT',(C,2*NB))
    gpad=P('gpad',(128,2*WIN)); gpadT=P('gpadT',(QT,128,GC)); osel=P('osel',(NCORES,1))
    xv=P('xvecs',(C,6)); sc=P('sconst',(1,8)); vps=P('vps',(C,3)); vpm=P('vpm',(C,1))
    mgw=P('m_gwT',(C,INTER)); prgw=P('pr_gwT',(3,C,INTER)); bagw=P('ba_gwT',(C,INTER))
    spgw=P('sp_gwT',(9*GC,INTER)); weT=P('w_effT',(2,128,9)); b6=P('bias128',(INTER,6))
    gf=P('gfill',(1,4*INTER)); mgbr=P('mgb_row',(1,INTER))
    out_ext=nc.declare_dram_parameter('out',[INTER,NB],F32,isOutput=True)

    with tile.TileContext(nc) as tc:
      with (tc.tile_pool(name="pp",bufs=1) as pp,
            tc.tile_pool(name="ww",bufs=4) as ww,
            tc.tile_pool(name="dr",bufs=1,space="DRAM") as dr,
            tc.tile_pool(name="ps_or",bufs=1,space="PSUM") as ps_or,
            tc.tile_pool(name="ps_six",bufs=1,space="PSUM") as ps_six,
            tc.tile_pool(name="ps_mid",bufs=2,space="PSUM") as ps_mid,
            tc.tile_pool(name="ps_roll",bufs=2,space="PSUM") as ps_roll,
            tc.tile_pool(name="ps_sm",bufs=1,space="PSUM") as ps_sm):
        dma=nc.sync.dma_start
        ag_in=dr.tile([AGS],F32); ag_out=dr.tile([NCORES*AGS],F32,addr_space='Shared')
        ar_in=dr.tile([ARS],F32); ar_out=dr.tile([ARS],F32,addr_space='Shared')
        p_glob=dr.tile([PGLOB],F32); p_loc=dr.tile([2816],F32)
        def ld(name,shape,src_ap):
            t=pp.tile(shape,F32,tag=name); dma(t[:],src_ap); return t
        xT_s=ld('xT',[128,KT,NB],xT.ap().rearrange("(k p) n -> p k n",p=128))
        yT_s=ld('yT',[128,3,KT,NB],yT.ap().rearrange("j (k p) n -> p j k n",p=128))
        gp_s=pp.tile([128,2,WIN],F32,tag='big',name='gp_s',padded_shape=[128,2,WIN])
        dma(gp_s[:],gpad.ap().rearrange("p (h w) -> p h w",h=2))
        xv_s=ld('xv',[128,KT,6],xv.ap().rearrange("(k p) n -> p k n",p=128))
        vp_s=ld('vp',[128,KT,3],vps.ap().rearrange("(k p) n -> p k n",p=128))
        vpm_s=ld('vpm',[128,KT,1],vpm.ap().rearrange("(k p) n -> p k n",p=128))
        mgw_s=ld('mgw',[128,KT,INTER],mgw.ap().rearrange("(k p) n -> p k n",p=128))
        pr_s=ld('pr',[128,3,KT,INTER],prgw.ap().rearrange("j (k p) n -> p j k n",p=128))
        bag_s=ld('bag',[128,KT,INTER],bagw.ap().rearrange("(k p) n -> p k n",p=128))
        spg_s=ld('spg',[128,18,INTER],spgw.ap().rearrange("(k p) n -> p k n",p=128))
        we_s=ld('we',[128,2,9],weT.ap().rearrange("h p n -> p h n"))
        b6_s=ld('b6',[INTER,6],b6.ap()); gf_s=ld('gf',[1,4*INTER],gf.ap())
        sc_s=ld('sc',[1,8],sc.ap()); mgbr_s=ld('mgbr',[1,INTER],mgbr.ap())
        osel_s=ld('osel',[NCORES,1],osel.ap())
        ones_c=pp.tile([128,1],F32,tag='ones_c'); nc.vector.memset(ones_c[:],1.0)
        zz=pp.tile([128,135],F32,tag='zz'); nc.vector.memset(zz[:],0.0)
        ONESR=gf_s[0:1,3*INTER:4*INTER]
        # conv -> b_s own rows
        outc=pp.tile([9,WIN],F32,tag='outc')
        for ch in range(5):
            pc=ps_mid.tile([128,512],F32,tag='mid')
            for h in range(2):
                nc.tensor.matmul(pc[:9,:468],we_s[:,h,:],gp_s[:,h,ch*468:(ch+1)*468],
                                 start=(h==0),stop=(h==1))
            nc.scalar.activation(outc[:,ch*468:(ch+1)*468],pc[:9,:468],AF.Copy)
        ov=outc[:].rearrange("p (h w) -> p h w",w=PW)
        bsa=pp.tile([HR,128],F32,tag='bsa')
        for m in range(9):
            kh,kw=divmod(m,3)
            bt=ww.tile([HR,128],F32,tag='bt')
            nc.sync.dma_start(bt[:],ov[m:m+1,kh:kh+HR,kw:kw+128])
            if m==0: nc.vector.tensor_copy(bsa[:],bt[:])
            else: nc.vector.tensor_tensor(bsa[:],bsa[:],bt[:],AL.add)
        dma(ag_in[NB*(INTER+1):AGS],bsa[:])
        # psum6
        p6=ps_six.tile([6,512],F32,tag='six')
        for kt in range(KT):
            nc.tensor.matmul(p6[:,:],xv_s[:,kt,:],xT_s[:,kt,:],start=(kt==0),
                             stop=(kt==KT-1))
        p6sb=pp.tile([6,512],F32,tag='p6sb')
        nc.scalar.activation(p6sb[:],p6[:,:],AF.Copy)
        p6r=[]
        for r in range(6):
            t=pp.tile([1,512],F32,tag=f'p6r{r}',name=f'p6r{r}')
            dma(t[:],p6sb[r:r+1,:]); p6r.append(t)
        s_sbs=[]
        for j in range(3):
            s_sbs.append(pp.tile([1,512],F32,tag=f's_sb{j}',name=f's_sb{j}'))
            psv=ps_mid.tile([128,512],F32,tag='mid')
            for kt in range(KT):
                nc.tensor.matmul(psv[:1,:],vp_s[:,kt,j:j+1],yT_s[:,j,kt,:],
                                 start=(kt==0),stop=(kt==KT-1))
            spre=ww.tile([1,512],F32,tag='spre',bufs=1)
            nc.vector.tensor_scalar(spre[:],psv[:1,:],sc_s[0:1,1+j:2+j],None,AL.add)
            t2=ww.tile([1,512],F32,tag='t2',bufs=1)
            nc.vector.tensor_tensor(t2[:],p6r[1+j][:],spre[:],AL.add)
            nc.scalar.activation(s_sbs[j][:],t2[:],AF.Relu)
        b_sb=pp.tile([1,512],F32,tag='b_sb')
        nc.vector.tensor_scalar(b_sb[:],p6r[5][:],sc_s[0:1,5:6],None,AL.add)
        dma(ag_in[NB*INTER:NB*(INTER+1)],b_sb[:])
        a_sb=pp.tile([1,512],F32,tag='a_sb')
        nc.vector.tensor_scalar(a_sb[:],p6r[0][:],sc_s[0:1,0:1],None,AL.add)
        # g_x row-major
        gxo=pp.tile([128,4,INTER],F32,tag='gxo')
        for i4 in range(4):
            pg=ps_mid.tile([128,512],F32,tag='mid')
            for kt in range(KT):
                nc.tensor.matmul(pg[:,:INTER],xT_s[:,kt,i4*128:(i4+1)*128],mgw_s[:,kt,:],
                                 start=(kt==0),stop=False,skip_group_check=True)
            nc.tensor.matmul(pg[:,:INTER],ONESR,mgbr_s[:],start=False,stop=True,
                             skip_group_check=True)
            nc.scalar.activation(gxo[:,i4,:],pg[:,:INTER],AF.Copy)
        dma(mkap(ag_in[:],0,[(128,128),(16384,4),(1,128)]),gxo[:])
        nc.gpsimd.collective_compute("AllGather",AL.bypass,ins=[ag_in[:].opt()],
                                     outs=[ag_out[:].opt()],replica_groups=RG)
        # softmax + p windows
        bs_f=pp.tile([128,128],F32,tag='bs_f')
        for c in range(NCORES):
            dma(bs_f[c*HR:(c+1)*HR,:],ag_out[c*AGS+NB*(INTER+1):c*AGS+AGS])
        e_sb=pp.tile([128,128],F32,tag='e_sb'); zc=pp.tile([128,1],F32,tag='zc')
        nc.scalar.activation(e_sb[:],bs_f[:],AF.Exp,accum_out=zc[:])
        pz=ps_sm.tile([128,512],F32,tag='sm')
        nc.tensor.matmul(pz[:1,:1],zc[:],ones_c[:],start=True,stop=True)
        z_sb=pp.tile([1,1],F32,tag='z_sb'); nc.vector.tensor_copy(z_sb[:],pz[:1,:1])
        zr=pp.tile([1,1],F32,tag='zr'); nc.vector.reciprocal(zr[:],z_sb[:])
        pzb=ps_sm.tile([128,512],F32,tag='sm')
        nc.tensor.matmul(pzb[:,:1],ONESR,zr[:],start=True,stop=True)
        zrb=pp.tile([128,1],F32,tag='zrb'); nc.vector.tensor_copy(zrb[:],pzb[:,:1])
        dma(p_glob[:],zz[:])
        dma(mkap(p_glob[:],262,[(130,128),(1,128)]),e_sb[:])
        p8=pp.tile([NCORES,2048],F32,tag='p8')
        dma(p8[:],mkap(p_glob[:],262,[(HR*PW,NCORES),(PW,HR),(1,128)]))
        ow_sb=pp.tile([1,2048],F32,tag='ow_sb')
        for ch in range(4):
            pwc=ps_sm.tile([128,512],F32,tag='sm')
            nc.tensor.matmul(pwc[:1,:512],osel_s[:],p8[:,ch*512:(ch+1)*512],
                             start=True,stop=True)
            nc.scalar.activation(ow_sb[:,ch*512:(ch+1)*512],pwc[:1,:512],AF.Copy)
        dma(p_loc[:],zz[:,:22])
        dma(mkap(p_loc[:],262,[(PW,HR),(1,128)]),ow_sb[:])
        # q matmuls
        pq=ps_mid.tile([128,512],F32,tag='mid')
        for t in range(QT):
            lq=ww.tile([128,9],F32,tag='lq')
            dma(lq[:],mkap(p_loc[:],128*t,[(1,128),(130,3),(1,3)]))
            gptt=ww.tile([128,GC],F32,tag='gptt',name=f'gptt{t}',bufs=2)
            dma(gptt[:],gpadT.ap()[t])
            nc.tensor.matmul(pq[:9,:GC],lq[:],gptt[:],start=(t==0),stop=(t==QT-1))
        q_sb=pp.tile([9,GC],F32,tag='q_sb')
        nc.scalar.activation(q_sb[:],pq[:9,:GC],AF.Copy)
        dma(ar_in[0:9*GC],q_sb[:])
        # colsums
        cs_sb=pp.tile([128,KT],F32,tag='cs_sb')
        bafv=bafT.ap().rearrange("(k p) n -> p k n",p=128)
        for kt in range(KT):
            bft=ww.tile([128,2*NB],F32,tag='bft',name=f'bft{kt}',bufs=2)
            dma(bft[:],bafv[:,kt,:])
            nc.vector.tensor_reduce(cs_sb[:,kt:kt+1],bft[:],
                                    axis=mybir.AxisListType.X,op=AL.add)
        dma(ar_in[9*GC:ARS],cs_sb[:].rearrange("p k -> k p"))
        nc.gpsimd.collective_compute("AllReduce",AL.add,ins=[ar_in[:].opt()],
                                     outs=[ar_out[:].opt()],replica_groups=RG)
        # post-AR small matvecs
        pgm=ps_sm.tile([128,512],F32,tag='sm2')
        ppm=ps_sm.tile([128,512],F32,tag='sm')
        for kt in range(KT):
            cst=ww.tile([128,1],F32,tag='cst')
            dma(cst[:],ar_out[9*GC+128*kt:9*GC+128*(kt+1)])
            nc.tensor.matmul(pgm[:,:1],bag_s[:,kt,:],cst[:],start=(kt==0),
                             stop=(kt==KT-1),skip_group_check=True)
            nc.tensor.matmul(ppm[:1,:1],vpm_s[:,kt,:],cst[:],start=(kt==0),
                             stop=(kt==KT-1),skip_group_check=True)
        gm_sb=pp.tile([128,1],F32,tag='gm_sb')
        nc.vector.tensor_scalar(gm_sb[:],pgm[:,:1],b6_s[:,4:5],None,AL.add)
        pm_sb=pp.tile([1,1],F32,tag='pm_sb')
        nc.vector.tensor_scalar(pm_sb[:],ppm[:1,:1],sc_s[0:1,4:5],None,AL.add)
        pv=ps_sm.tile([128,512],F32,tag='sm2')
        for t in range(18):
            qrt=ww.tile([128,1],F32,tag='qrt')
            dma(qrt[:],ar_out[128*t:128*(t+1)])
            nc.tensor.matmul(pv[:,:1],spg_s[:,t,:],qrt[:],start=(t==0),stop=(t==17))
        v_sb=pp.tile([128,1],F32,tag='v_sb')
        nc.vector.tensor_scalar(v_sb[:],pv[:,:1],zrb[:],b6_s[:,5:6],AL.mult,AL.add)
        # s_ba + broadcasts + pair terms
        sba=pp.tile([1,512],F32,tag='sba')
        nc.scalar.activation(sba[:],p6r[4][:],AF.Relu,bias=pm_sb[0:1,0:1])
        acc=pp.tile([128,512],F32,tag='acc')
        tmp=pp.tile([128,512],F32,tag='tmp')
        for j in range(3):
            py=ps_roll.tile([128,512],F32,tag='roll')
            for kt in range(KT):
                nc.tensor.matmul(py[:,:],pr_s[:,j,kt,:],yT_s[:,j,kt,:],
                                 start=(kt==0),stop=(kt==KT-1))
            gy=ww.tile([128,512],F32,tag='gy',bufs=1)
            nc.vector.tensor_scalar(gy[:],py[:,:],b6_s[:,1+j:2+j],None,AL.add)
            pb=ps_roll.tile([128,512],F32,tag='roll')
            nc.tensor.matmul(pb[:,:],gf_s[0:1,j*INTER:(j+1)*INTER],s_sbs[j][:],
                             start=True,stop=True)
            if j==0:
                nc.vector.tensor_tensor(acc[:],gy[:],pb[:,:],AL.mult)
            else:
                nc.vector.tensor_tensor(tmp[:],gy[:],pb[:,:],AL.mult)
                nc.vector.tensor_tensor(acc[:],acc[:],tmp[:],AL.add)
        psb=ps_roll.tile([128,512],F32,tag='roll')
        nc.tensor.matmul(psb[:,:],ONESR,sba[:],start=True,stop=True)
        nc.vector.tensor_scalar(tmp[:],psb[:,:],gm_sb[:],None,AL.mult)
        nc.vector.tensor_tensor(acc[:],acc[:],tmp[:],AL.add)
        pab=ps_roll.tile([128,512],F32,tag='roll')
        nc.tensor.matmul(pab[:,:],ONESR,a_sb[:],start=True,stop=True)
        ab_sb=pp.tile([128,512],F32,tag='ab_sb')
        nc.scalar.activation(ab_sb[:],pab[:,:],AF.Copy)
        # gx readback + origin loop
        gx_sb=pp.tile([128,32,128],F32,tag='big',name='gx_sb')
        for c in range(NCORES):
            dma(gx_sb[:,4*c:4*(c+1),:],mkap(ag_out[:],c*AGS,[(128,128),(16384,4),(1,128)]))
        po=ps_or.tile([128,512],F32,tag='orig')
        for jt in range(32):
            cc,lt=divmod(jt,4)
            bc=ww.tile([128,1],F32,tag='bc')
            dma(bc[:],ag_out[cc*AGS+NB*INTER+lt*128:cc*AGS+NB*INTER+(lt+1)*128])
            fT=ww.tile([128,512],F32,tag='fT',bufs=2)
            if jt%8<3:
                nc.scalar.activation(fT[:],ab_sb[:],AF.Relu,bias=bc[:])
            else:
                nc.vector.tensor_scalar(fT[:],ab_sb[:],bc[:],0.0,AL.add,AL.max)
            nc.tensor.matmul(po[:,:],gx_sb[:,jt,:],fT[:],start=(jt==0),stop=(jt==31))
        ot=pp.tile([128,512],F32,tag='ot')
        nc.vector.tensor_scalar(ot[:],po[:,:],1.0/N,v_sb[:],AL.mult,AL.add)
        fin=pp.tile([128,512],F32,tag='fin')
        nc.vector.tensor_tensor(fin[:],acc[:],ot[:],AL.add)
        dma(out_ext.ap(),fin[:])
    nc.compile()
    return nc

